# revision 1
# baseline (speedup 1.0000x reference)
"""Trainium2 Bass kernel for nn_Model4 (5-layer GCN message passing).

Strategy (8 NeuronCores, SPMD):
  - Nodes sharded row-wise: core i owns sources [12500*i, 12500*(i+1)).
  - GCN normalization folded into node features:
      h' = dinv * ([a|b] @ Wg.T) = ([dinv*a | dinv*b]) @ Wg.T
      a_new[c] = relu(dinv[c] * (sum_{e in(c)} h'[src_e] + h'[c]) + bg)
    (the +h'[c] term is the PyG self-loop).
  - Per step, each core gathers h'[src] for its own edges from an
    SBUF-resident transposed feature table (features on partitions,
    replicated per 16-partition GPSIMD core group; each of the 8 Q7
    cores processes edges destined to one node-range band), computes
    destination segment sums via fp32 cumulative sum (DVE scan) +
    boundary gather + shifted subtract, and the 8 cores' partial sums
    are combined with a ReduceScatter so each core receives the full
    aggregation for its own nodes.
  - Edge index lists / segment boundaries are precomputed host-side
    (static graph metadata), wrapped in the Q7 16-partition layout.
  - Final scalar: per-core partial readout sums, combined on host:
    tanh((sum_i part_i + N*b3) / N).
"""
import numpy as np

import concourse.bacc as bacc
import concourse.mybir as mybir
import concourse.tile as tile
from concourse.bass_utils import run_bass_kernel_spmd

N = 100000
NCORES = 8
NPC = N // NCORES              # 12500 sources per core / dests per band
PADN = 12544                   # padded node axis (98 * 128)
NCH = 8                        # dest chunks per band
DCH = PADN // NCH              # 1568
BN = DCH + 32                  # 1600 boundary idxs; mult of 32 so per-chunk
                               # int16 idx slices stay 4B-aligned for Q7 ucode
NSTEP = 5
CW = 448                       # node-chunk width for update phase
NCHK = PADN // CW              # 28
F32 = mybir.dt.float32
I16 = mybir.dt.int16
AX = mybir.AluOpType
ACTF = mybir.ActivationFunctionType

_cache = {}


def _preprocess(edges):
    row = np.ascontiguousarray(edges[0]).astype(np.int64)
    col = np.ascontiguousarray(edges[1]).astype(np.int64)
    core = row // NPC
    band = col // NPC
    ld = (col - band * NPC).astype(np.int32)
    ls = (row - core * NPC).astype(np.int32)
    chunk = ld // DCH
    key = (core * 8 + band) * 8 + chunk
    order = np.lexsort((ld, key))
    key_s = key[order]
    ld_s = ld[order]
    ls_s = ls[order]
    starts = np.searchsorted(key_s, np.arange(NCORES * NCORES * NCH + 1))
    sizes = np.diff(starts)
    ec = int(sizes.max())
    EC = max(((ec + 31) // 32) * 32, 32)
    assert EC + 1 < 32768

    eidx = np.zeros((NCORES, NCH, 128, EC // 16), dtype=np.int16)
    bidx = np.zeros((NCORES, NCH, 128, BN // 16), dtype=np.int16)
    dr = np.arange(DCH + 1, dtype=np.int64)
    for i in range(NCORES):
        for k in range(NCORES):
            for c in range(NCH):
                b = (i * 8 + k) * 8 + c
                b0, b1 = starts[b], starts[b + 1]
                n = b1 - b0
                flat = np.zeros(EC, dtype=np.int16)
                flat[:n] = ls_s[b0:b1].astype(np.int16)
                eidx[i, c, 16 * k:16 * (k + 1)] = flat.reshape(EC // 16, 16).T
                bnd = np.searchsorted(ld_s[b0:b1], c * DCH + dr, side="left")
                flatb = np.zeros(BN, dtype=np.int16)
                flatb[:DCH + 1] = bnd.astype(np.int16)
                bidx[i, c, 16 * k:16 * (k + 1)] = flatb.reshape(BN // 16, 16).T

    # pre-transpose to the SBUF layout [128, NCH, S] so the device DMA is
    # a single contiguous copy (a strided transposing DMA costs ~300us)
    eidx = np.ascontiguousarray(eidx.transpose(0, 2, 1, 3))
    bidx = np.ascontiguousarray(bidx.transpose(0, 2, 1, 3))
    deg = (np.bincount(col, minlength=N) + 1).astype(np.float32)
    dinv = deg ** np.float32(-0.5)
    return eidx, bidx, dinv, EC


def _build(EC):
    nc = bacc.Bacc("TRN2", target_bir_lowering=False, debug=False,
                   num_devices=NCORES)

    xin_d = nc.dram_tensor("xin", [19, PADN], F32, kind="ExternalInput")
    dinv_d = nc.dram_tensor("dinv", [19, PADN], F32, kind="ExternalInput")
    eidx_d = nc.dram_tensor("eidx", [128, NCH, EC // 16], I16, kind="ExternalInput")
    bidx_d = nc.dram_tensor("bidx", [128, NCH, BN // 16], I16, kind="ExternalInput")
    w1_d = nc.dram_tensor("w1", [15, 15], F32, kind="ExternalInput")
    wga_d = nc.dram_tensor("wga", [15, 15], F32, kind="ExternalInput")
    wgb_d = nc.dram_tensor("wgb", [19, 15], F32, kind="ExternalInput")
    w4_d = nc.dram_tensor("w4", [19, 19], F32, kind="ExternalInput")
    w3a_d = nc.dram_tensor("w3a", [15, 1], F32, kind="ExternalInput")
    w3b_d = nc.dram_tensor("w3b", [19, 1], F32, kind="ExternalInput")
    b1_d = nc.dram_tensor("b1", [15, 1], F32, kind="ExternalInput")
    bg_d = nc.dram_tensor("bg", [15, 1], F32, kind="ExternalInput")
    b4_d = nc.dram_tensor("b4", [19, 1], F32, kind="ExternalInput")

    part_d = nc.dram_tensor("part", [1, 1], F32, kind="ExternalOutput")

    rs_in = nc.dram_tensor("rs_in", [128, PADN], F32)
    rs_out = nc.dram_tensor("rs_out", [16, PADN], F32)
    bt_dram = nc.dram_tensor("bt_dram", [19, PADN], F32)

    with tile.TileContext(nc) as tc:
        with (
            tc.tile_pool(name="static", bufs=1) as st,
            tc.tile_pool(name="gbuf", bufs=2) as gbuf,
            tc.tile_pool(name="ebuf", bufs=1) as ebuf,
            tc.tile_pool(name="nbuf", bufs=3) as nbuf,
            tc.tile_pool(name="ps", bufs=1, space="PSUM") as ps,
            tc.tile_pool(name="ps2", bufs=3, space="PSUM") as ps2,
        ):
            TAB = st.tile([128, PADN], F32)
            EIDX = st.tile([128, NCH, EC // 16], I16)
            BIDX = st.tile([128, NCH, BN // 16], I16)
            w1 = st.tile([15, 15], F32)
            wga = st.tile([15, 15], F32)
            wgb = st.tile([19, 15], F32)
            w4 = st.tile([19, 19], F32)
            w3a = st.tile([15, 1], F32)
            w3b = st.tile([19, 1], F32)
            b1 = st.tile([15, 1], F32)
            bg = st.tile([15, 1], F32)
            b4 = st.tile([19, 1], F32)
            acc = st.tile([1, 1], F32)

            nc.vector.memset(TAB[:], 0.0)
            nc.vector.memset(acc[:], 0.0)
            nc.sync.dma_start(out=EIDX[:], in_=eidx_d[:])
            nc.sync.dma_start(out=BIDX[:], in_=bidx_d[:])
            for t, d in ((w1, w1_d), (wga, wga_d), (wgb, wgb_d), (w4, w4_d),
                         (w3a, w3a_d), (w3b, w3b_d), (b1, b1_d), (bg, bg_d),
                         (b4, b4_d)):
                nc.sync.dma_start(out=t[:], in_=d[:])

            def readout(psr_tile, cols):
                red = nbuf.tile([1, 1], F32, tag="red")
                nc.vector.tensor_reduce(
                    out=red[:], in_=psr_tile[0:1, 0:cols],
                    axis=mybir.AxisListType.X, op=AX.add)
                nc.vector.tensor_add(acc[:], acc[:], red[:])

            # ---- init: b, b~, a0, a0~, h'0 ----
            for j in range(NCHK):
                sl = slice(CW * j, CW * (j + 1))
                real = max(min(NPC - CW * j, CW), 0)
                xin_t = nbuf.tile([19, CW], F32, tag="xin")
                dv_t = nbuf.tile([19, CW], F32, tag="dv")
                nc.sync.dma_start(out=xin_t[:], in_=xin_d[:, sl])
                nc.scalar.dma_start(out=dv_t[:], in_=dinv_d[:, sl])
                psb = ps.tile([19, CW], F32, tag="ps19")
                nc.tensor.matmul(out=psb[:], lhsT=w4[:], rhs=xin_t[:],
                                 start=True, stop=True)
                bt_t = nbuf.tile([19, CW], F32, tag="bt")
                nc.scalar.activation(out=bt_t[:], in_=psb[:],
                                     func=ACTF.Identity, bias=b4[:], scale=1.0)
                if real > 0:
                    psr = ps.tile([1, CW], F32, tag="ps1")
                    nc.tensor.matmul(out=psr[:], lhsT=w3b[:], rhs=bt_t[:],
                                     start=True, stop=True)
                    readout(psr, real)
                btt = nbuf.tile([19, CW], F32, tag="btt")
                nc.vector.tensor_mul(btt[:], bt_t[:], dv_t[:])
                nc.sync.dma_start(out=bt_dram[:, sl], in_=btt[:])
                psa = ps2.tile([15, CW], F32, tag="ps150")
                nc.tensor.matmul(out=psa[:], lhsT=w1[:], rhs=xin_t[0:15, :],
                                 start=True, stop=True)
                a_t = nbuf.tile([15, CW], F32, tag="a")
                nc.scalar.activation(out=a_t[:], in_=psa[:],
                                     func=ACTF.Relu, bias=b1[:], scale=1.0)
                at_t = nbuf.tile([15, CW], F32, tag="at")
                nc.vector.tensor_mul(at_t[:], a_t[:], dv_t[0:15, :])
                psh = ps2.tile([15, CW], F32, tag="ps151")
                nc.tensor.matmul(out=psh[:], lhsT=wga[:], rhs=at_t[:],
                                 start=True, stop=False)
                nc.tensor.matmul(out=psh[:], lhsT=wgb[:], rhs=btt[:],
                                 start=False, stop=True)
                nc.scalar.copy(out=TAB[0:15, sl], in_=psh[:])
            for k in range(1, 8):
                nc.sync.dma_start(out=TAB[16 * k:16 * (k + 1), :],
                                  in_=TAB[0:16, :])

            # ---- 5 GCN steps ----
            for s in range(NSTEP):
                # software-pipelined emission: edge-gather c+1 is issued
                # before chunk c's boundary gather so the Q7 cores never
                # idle waiting for the DVE scan.
                gs = {}

                def issue_gather(c, gs=gs):
                    G = gbuf.tile([128, EC], F32, tag="G")
                    nc.gpsimd.ap_gather(
                        out_ap=G[:], in_ap=TAB[:], idxs_ap=EIDX[:, c, :],
                        channels=128, num_elems=PADN, d=1, num_idxs=EC)
                    gs[c] = G

                issue_gather(0)
                for c in range(NCH):
                    if c + 1 < NCH:
                        issue_gather(c + 1)
                    G = gs.pop(c)
                    P = ebuf.tile([128, EC + 1], F32, tag="P")
                    nc.vector.memset(P[:, 0:1], 0.0)
                    nc.vector.tensor_tensor_scan(
                        out=P[:, 1:EC + 1], data0=G[:], data1=G[:],
                        initial=0.0, op0=AX.add, op1=AX.bypass)
                    B = ebuf.tile([128, BN], F32, tag="B")
                    nc.gpsimd.ap_gather(
                        out_ap=B[:], in_ap=P[:], idxs_ap=BIDX[:, c, :],
                        channels=128, num_elems=EC + 1, d=1, num_idxs=BN)
                    S = ebuf.tile([128, DCH], F32, tag="S")
                    nc.vector.tensor_sub(S[:], B[:, 1:DCH + 1], B[:, 0:DCH])
                    nc.sync.dma_start(out=rs_in[:, DCH * c:DCH * (c + 1)],
                                      in_=S[:])
                nc.gpsimd.collective_compute(
                    "ReduceScatter", AX.add,
                    replica_groups=[list(range(NCORES))],
                    ins=[rs_in[:]], outs=[rs_out[:]])
                last = s == NSTEP - 1
                UW, MW = 784, 392          # update chunk / matmul sub-width
                for j in range(PADN // UW):
                    sl = slice(UW * j, UW * (j + 1))
                    real = max(min(NPC - UW * j, UW), 0)
                    st_t = nbuf.tile([15, UW], F32, tag="st")
                    dv_t = nbuf.tile([15, UW], F32, tag="dv")
                    nc.sync.dma_start(out=st_t[:], in_=rs_out[0:15, sl])
                    nc.scalar.dma_start(out=dv_t[:], in_=dinv_d[0:15, sl])
                    t1 = nbuf.tile([15, UW], F32, tag="t1")
                    nc.vector.tensor_add(t1[:], st_t[:], TAB[0:15, sl])
                    nc.vector.tensor_mul(t1[:], t1[:], dv_t[0:15, :])
                    a_t = nbuf.tile([15, UW], F32, tag="a")
                    nc.scalar.activation(out=a_t[:], in_=t1[:],
                                         func=ACTF.Relu, bias=bg[:], scale=1.0)
                    if last:
                        for q in range(2):
                            rq = max(min(real - MW * q, MW), 0)
                            if rq > 0:
                                psr = ps.tile([1, MW], F32, tag="ps1")
                                nc.tensor.matmul(
                                    out=psr[:], lhsT=w3a[:],
                                    rhs=a_t[:, MW * q:MW * (q + 1)],
                                    start=True, stop=True)
                                readout(psr, rq)
                    else:
                        nc.vector.tensor_mul(a_t[:], a_t[:], dv_t[0:15, :])
                        btt = nbuf.tile([19, UW], F32, tag="btt")
                        nc.scalar.dma_start(out=btt[:], in_=bt_dram[:, sl])
                        for q in range(2):
                            qs = slice(MW * q, MW * (q + 1))
                            psh = ps2.tile([15, MW], F32, tag=f"ps15{q}")
                            nc.tensor.matmul(out=psh[:], lhsT=wga[:],
                                             rhs=a_t[:, qs],
                                             start=True, stop=False)
                            nc.tensor.matmul(out=psh[:], lhsT=wgb[:],
                                             rhs=btt[:, qs],
                                             start=False, stop=True)
                            nc.scalar.copy(
                                out=TAB[0:15, UW * j + MW * q:
                                        UW * j + MW * (q + 1)],
                                in_=psh[:])
                if not last:
                    for k in range(1, 8):
                        nc.sync.dma_start(out=TAB[16 * k:16 * (k + 1), :],
                                          in_=TAB[0:16, :])

            nc.sync.dma_start(out=part_d[:], in_=acc[:])
    nc.finalize()
    return nc


def kernel(**inputs):
    x1 = np.ascontiguousarray(np.asarray(inputs["x1"], dtype=np.float32))
    x2 = np.ascontiguousarray(np.asarray(inputs["x2"], dtype=np.float32))
    edges = np.asarray(inputs["edges"])
    W1 = np.asarray(inputs["W1"], dtype=np.float32)
    b1 = np.asarray(inputs["b1"], dtype=np.float32)
    Wg = np.asarray(inputs["Wg"], dtype=np.float32)
    bg = np.asarray(inputs["bg"], dtype=np.float32)
    W3 = np.asarray(inputs["W3"], dtype=np.float32)
    b3 = np.asarray(inputs["b3"], dtype=np.float32)
    W4 = np.asarray(inputs["W4"], dtype=np.float32)
    b4 = np.asarray(inputs["b4"], dtype=np.float32)

    eidx, bidx, dinv, EC = _preprocess(edges)

    x2t = np.tile(x2, (20, 1))
    xin = np.concatenate([x1, x2t], axis=1)     # [N, 19]

    in_maps = []
    for i in range(NCORES):
        sl = slice(NPC * i, NPC * (i + 1))
        xinT = np.zeros((19, PADN), dtype=np.float32)
        xinT[:, :NPC] = xin[sl].T
        dvT = np.zeros((19, PADN), dtype=np.float32)
        dvT[:, :NPC] = np.broadcast_to(dinv[sl], (19, NPC))
        in_maps.append(dict(
            xin=xinT, dinv=dvT,
            eidx=eidx[i], bidx=bidx[i],
            w1=np.ascontiguousarray(W1.T),
            wga=np.ascontiguousarray(Wg[:, :15].T),
            wgb=np.ascontiguousarray(Wg[:, 15:].T),
            w4=np.ascontiguousarray(W4.T),
            w3a=np.ascontiguousarray(W3[0, :15, None]),
            w3b=np.ascontiguousarray(W3[0, 15:, None]),
            b1=np.ascontiguousarray(b1[:, None]),
            bg=np.ascontiguousarray(bg[:, None]),
            b4=np.ascontiguousarray(b4[:, None]),
        ))

    key = EC
    if key not in _cache:
        _cache[key] = _build(EC)
    nc = _cache[key]

    res = run_bass_kernel_spmd(nc, in_maps, list(range(NCORES))).results
    total = sum(float(res[i]["part"][0, 0]) for i in range(NCORES))
    out = np.tanh((total + N * float(b3.reshape(-1)[0])) / N)
    return np.float32(out)

